# revision 12
# baseline (speedup 1.0000x reference)
"""Multi-head attention (B=2, L=2048, H=1024, NH=16) on 8 TRN2 NeuronCores.

Sharding: data-parallel over batch (2) x tensor-parallel over heads (4 groups
of 4 heads).  core = b*4 + g handles batch b, heads [4g, 4g+4).  Wq/Wk/Wv are
split column-wise, Wo row-wise; each core produces a partial [L, H] output
that the host sums per batch (the row-parallel all-reduce done host-side).

Device math (per core), all matmuls bf16 inputs / fp32 PSUM accumulation:
  QT = (Wq*0.125)^T x^T          [256, 2048]  (softmax scale folded into Wq)
  KT = Wk^T y^T                  [256, 2048]
  V  = y Wv                      [2048, 256] stored as V_aug [lk, 4*(64+1)]
                                 with a ones column per head
  per head h, per lq chunk:
    S^T[lk, lq] = KT_h^T QT_h    (contraction d=64; head pairs packed at
                                  partition offsets 0/64 -> PE row tiling)
    P^T = exp(S^T)               ScalarE, PSUM -> SBUF bf16
    O^T_aug[65, lq] = sum_lk V_aug_h^T P^T   (row 64 = softmax denominators)
    O'^T = O^T * broadcast(1/sums)           DVE recip + GpSimd partition bcast
  out[lq, 1024] += O'^T_cat^T Wo  (partial; host sums the 4 head-groups)
"""

import numpy as np
import ml_dtypes

B, L, H, NH, D = 2, 2048, 1024, 16, 64
GP = 4            # head-groups (tensor-parallel factor)
CH = H // GP      # 256 local projection cols per core
HL = NH // GP     # 4 local heads
LQ = 1024         # lq chunk size
NLQ = L // LQ
NKT = L // 128    # 16 lk tiles
BF16 = ml_dtypes.bfloat16

_CACHE = {}


def _build():
    import concourse.mybir as mybir
    import concourse.tile as tile
    from concourse import bacc

    dt = mybir.dt
    f32, bf16 = dt.float32, dt.bfloat16
    Exp = mybir.ActivationFunctionType.Exp

    nc = bacc.Bacc("TRN2", target_bir_lowering=False, debug=False)
    xT = nc.declare_dram_parameter("xT", [H, L], bf16, isOutput=False)
    yT = nc.declare_dram_parameter("yT", [H, L], bf16, isOutput=False)
    wq = nc.declare_dram_parameter("wq", [H, CH], bf16, isOutput=False)
    wk = nc.declare_dram_parameter("wk", [H, CH], bf16, isOutput=False)
    wv = nc.declare_dram_parameter("wv", [H, CH], bf16, isOutput=False)
    wo = nc.declare_dram_parameter("wo", [CH, H], bf16, isOutput=False)
    out = nc.declare_dram_parameter("out", [L, H], f32, isOutput=True)

    with tile.TileContext(nc) as tc:
        with (
            tc.tile_pool(name="w", bufs=1) as wpool,
            tc.tile_pool(name="acts", bufs=1) as apool,
            tc.tile_pool(name="psA", bufs=2, space="PSUM") as psA,
            tc.tile_pool(name="psO", bufs=2, space="PSUM") as psO,
            tc.tile_pool(name="pt", bufs=18) as ptpool,
            tc.tile_pool(name="oT", bufs=2) as otpool,
            tc.tile_pool(name="sm", bufs=2) as smpool,
            tc.tile_pool(name="osb", bufs=4) as opool,
        ):
            # prefetch the exp activation table while input DMAs run
            dummy = smpool.tile([1, 8], f32, tag="dummy")
            nc.vector.memset(dummy, 0.0)
            nc.scalar.activation(dummy, dummy, Exp)

            # ---- input DMAs: big sprayed transfers (full queue fan-out),
            # ordered so heads 0/1 of chunk 0 can start earliest ----------
            wq_sb = wpool.tile([128, 8, CH], bf16, tag="wq")
            nc.sync.dma_start(wq_sb, wq.rearrange("(t p) c -> p t c", p=128))
            xT_sb = apool.tile([128, 8, L], bf16, tag="xT")
            xr = xT.rearrange("(t p) l -> p t l", p=128)
            nc.sync.dma_start(xT_sb[:, :, 0:LQ], xr[:, :, 0:LQ])
            wk_sb = wpool.tile([128, 8, CH], bf16, tag="wk")
            nc.sync.dma_start(wk_sb, wk.rearrange("(t p) c -> p t c", p=128))
            yT_sb = apool.tile([128, 8, L], bf16, tag="yT")
            yr = yT.rearrange("(t p) l -> p t l", p=128)
            nc.sync.dma_start(yT_sb[:, :, 0:LQ], yr[:, :, 0:LQ])
            nc.sync.dma_start(yT_sb[:, :, LQ:L], yr[:, :, LQ:L])
            nc.sync.dma_start(xT_sb[:, :, LQ:L], xr[:, :, LQ:L])
            wv_sb = wpool.tile([128, 8, CH], bf16, tag="wv")
            nc.sync.dma_start(wv_sb, wv.rearrange("(t p) c -> p t c", p=128))
            wo_sb = wpool.tile([128, 2, H], bf16, tag="wo")
            nc.sync.dma_start(wo_sb, wo.rearrange("(t p) c -> p t c", p=128))

            qT_sb = apool.tile([128, 2, L], bf16, tag="qT")
            kT_sb = apool.tile([128, 2, L], bf16, tag="kT")
            vaug_sb = apool.tile([128, NKT, HL * 65], bf16, tag="vaug")

            def proj(w_sb, act_sb, dst, ct):
                # dst[:, ct, :] = (W chunk)^T @ actT   [128, L]
                for lh in range(L // LQ):
                    ps = psA.tile([128, LQ], f32, tag="psA")
                    for ht in range(8):
                        for sl in range(LQ // 512):
                            nc.tensor.matmul(
                                ps[:, sl * 512:(sl + 1) * 512],
                                lhsT=w_sb[:, ht, ct * 128:(ct + 1) * 128],
                                rhs=act_sb[:, ht, lh * LQ + sl * 512:
                                           lh * LQ + (sl + 1) * 512],
                                start=(ht == 0), stop=(ht == 7),
                            )
                    nc.vector.tensor_copy(dst[:, ct, lh * LQ:(lh + 1) * LQ], ps)

            def v_proj():
                # V_aug[lk, 4*(64+1)] bf16 with a ones column per head
                for lkt in range(NKT):
                    psv = psA.tile([128, LQ], f32, tag="psA")
                    for ht in range(8):
                        nc.tensor.matmul(
                            psv[:, :CH],
                            lhsT=yT_sb[:, ht, lkt * 128:(lkt + 1) * 128],
                            rhs=wv_sb[:, ht, :],
                            start=(ht == 0), stop=(ht == 7),
                        )
                    vh = vaug_sb[:, lkt, :].rearrange("p (h e) -> p h e", h=HL)
                    nc.vector.tensor_copy(
                        vh[:, :, 0:64],
                        psv[:, :CH].rearrange("p (h e) -> p h e", h=HL))
                    nc.vector.memset(vh[:, :, 64], 1.0)

            def s2a(ci, h):
                # S^T = K_h^T^T Q_h^T per lk tile; exp -> P^T bf16 tiles
                po, ct2 = h % 2, h // 2
                pts = []
                for lkt in range(NKT):
                    psS = psA.tile([128, LQ], f32, tag="psA")
                    for sl in range(LQ // 512):
                        nc.tensor.matmul(
                            psS[:, sl * 512:(sl + 1) * 512],
                            lhsT=kT_sb[64 * po:64 * po + 64, ct2,
                                       lkt * 128:(lkt + 1) * 128],
                            rhs=qT_sb[64 * po:64 * po + 64, ct2,
                                      ci * LQ + sl * 512:ci * LQ + (sl + 1) * 512],
                            start=True, stop=True,
                        )
                    pt = ptpool.tile([128, LQ], bf16, tag="pt")
                    nc.scalar.activation(pt, psS, Exp)
                    pts.append(pt)
                return pts

            def s2b(ci, h, pts, oT_sb):
                # O^T_aug[65, LQ] = sum_lk V_aug_h^T P^T; then normalize
                po, ct2 = h % 2, h // 2
                psO_h = psO.tile([128, LQ], f32, tag="psO")
                for lkt in range(NKT):
                    for sl in range(LQ // 512):
                        nc.tensor.matmul(
                            psO_h[0:65, sl * 512:(sl + 1) * 512],
                            lhsT=vaug_sb[:, lkt, h * 65:(h + 1) * 65],
                            rhs=pts[lkt][:, sl * 512:(sl + 1) * 512],
                            start=(lkt == 0), stop=(lkt == NKT - 1),
                        )
                # copy PSUM->SBUF first (frees the psO slot in one DVE pass);
                # recip/bcast/mul then run off the PSUM critical path
                ocp = smpool.tile([64, LQ], f32, tag="ocp")
                nc.vector.tensor_copy(ocp, psO_h[0:64, :])
                sums = smpool.tile([1, LQ], f32, tag="sums")
                nc.vector.tensor_copy(sums, psO_h[64:65, :])
                # NB: reciprocal_approx_fast misbehaves when its input AP has
                # a non-zero base partition, hence the sums copy.
                recip = smpool.tile([1, LQ], f32, tag="recip")
                nc.vector.reciprocal_approx_fast(recip, sums)
                bcast = smpool.tile([64, LQ], f32, tag="bcast")
                nc.gpsimd.partition_broadcast(bcast, recip)
                nc.vector.tensor_mul(
                    oT_sb[64 * po:64 * po + 64, ct2, :], ocp, bcast)

            def s2(ci, h, oT_sb):
                s2b(ci, h, s2a(ci, h), oT_sb)

            def s3(ci, oT_sb):
                # out[lq, :] = O'^T_cat^T Wo (partial over this core's heads)
                for mt in range(LQ // 128):
                    for nt in range(2):
                        pso = psO.tile([128, LQ], f32, tag="psO")
                        for kt in range(2):
                            nc.tensor.matmul(
                                pso[:, :512],
                                lhsT=oT_sb[:, kt, mt * 128:(mt + 1) * 128],
                                rhs=wo_sb[:, kt, nt * 512:(nt + 1) * 512],
                                start=(kt == 0), stop=(kt == 1),
                            )
                        osb = opool.tile([128, 512], f32, tag="osb")
                        nc.vector.tensor_copy(osb, pso[:, :512])
                        nc.sync.dma_start(
                            out[ci * LQ + mt * 128:ci * LQ + (mt + 1) * 128,
                                nt * 512:(nt + 1) * 512],
                            osb)

            # ---- emission order: start the exp stream as early as possible
            # (heads 0/1 only need the ct0 halves of Q^T/K^T), and software-
            # pipeline stage 3 one chunk behind stage 2 --------------------
            oT = [otpool.tile([128, 2, LQ], bf16, tag="oT", name=f"oT{i}")
                  for i in range(NLQ)]
            proj(wq_sb, xT_sb, qT_sb, 0)
            proj(wk_sb, yT_sb, kT_sb, 0)
            pts00 = s2a(0, 0)
            v_proj()
            s2b(0, 0, pts00, oT[0])
            s2(0, 1, oT[0])
            proj(wq_sb, xT_sb, qT_sb, 1)
            proj(wk_sb, yT_sb, kT_sb, 1)
            s2(0, 2, oT[0])
            s2(0, 3, oT[0])
            s2(1, 0, oT[1])
            s3(0, oT[0])
            s2(1, 1, oT[1])
            s2(1, 2, oT[1])
            s2(1, 3, oT[1])
            s3(1, oT[1])
    nc.compile()
    return nc


def _get_nc():
    if "nc" not in _CACHE:
        _CACHE["nc"] = _build()
    return _CACHE["nc"]


def _in_maps(x, y, Wq, Wk, Wv, Wo):
    maps = []
    for core in range(8):
        b, g = core // GP, core % GP
        cs = slice(g * CH, (g + 1) * CH)
        maps.append({
            "xT": np.ascontiguousarray(x[b].T).astype(BF16),
            "yT": np.ascontiguousarray(y[b].T).astype(BF16),
            "wq": np.ascontiguousarray(Wq[:, cs] * np.float32(0.125)).astype(BF16),
            "wk": np.ascontiguousarray(Wk[:, cs]).astype(BF16),
            "wv": np.ascontiguousarray(Wv[:, cs]).astype(BF16),
            "wo": np.ascontiguousarray(Wo[cs, :]).astype(BF16),
        })
    return maps


def _install_ntff_hook():
    """Provide the antenv.axon_hooks shim missing from this container so
    run_bass_kernel_spmd(trace=True) can drive NTFF profiling via ctypes."""
    import sys
    import types
    try:
        from antenv.axon_hooks import get_axon_ntff_profile_hook  # noqa: F401
        return
    except ImportError:
        pass
    from trn_agent_boot.trn_boot import _ntff_profile_via_ctypes
    hook = _ntff_profile_via_ctypes("/opt/axon/libaxon_pjrt.so")
    mod = types.ModuleType("antenv.axon_hooks")
    mod.get_axon_ntff_profile_hook = lambda: hook
    mod.set_axon_ntff_profile_hook = lambda h: None
    sys.modules["antenv.axon_hooks"] = mod


def _run(inputs, trace=False):
    from concourse import bass_utils

    if trace:
        _install_ntff_hook()

    x, y, bias = inputs["x"], inputs["y"], inputs["bias"]
    if np.count_nonzero(np.asarray(bias)):
        raise NotImplementedError("nonzero attention bias not supported")
    nc = _get_nc()
    maps = _in_maps(np.asarray(x, np.float32), np.asarray(y, np.float32),
                    np.asarray(inputs["Wq"], np.float32),
                    np.asarray(inputs["Wk"], np.float32),
                    np.asarray(inputs["Wv"], np.float32),
                    np.asarray(inputs["Wo"], np.float32))
    res = bass_utils.run_bass_kernel_spmd(
        nc, maps, list(range(8)), trace=trace)
    out = np.zeros((B, L, H), np.float32)
    for core in range(8):
        out[core // GP] += res.results[core]["out"]
    return out, res


def kernel(**inputs):
    out, _ = _run(inputs, trace=False)
    return out


# revision 15
# speedup vs baseline: 1.1432x; 1.1432x over previous
"""Multi-head attention (B=2, L=2048, H=1024, NH=16) on 8 TRN2 NeuronCores.

Sharding: data-parallel over batch (2) x tensor-parallel over heads (4 groups
of 4 heads).  core = b*4 + g handles batch b, heads [4g, 4g+4).  Wq/Wk/Wv are
split column-wise, Wo row-wise; each core produces a partial [L, H] output
that the host sums per batch (the row-parallel all-reduce done host-side).

Device math (per core), all matmuls bf16 inputs / fp32 PSUM accumulation:
  QT = (Wq*0.125)^T x^T          [256, 2048]  (softmax scale folded into Wq)
  KT = Wk^T y^T                  [256, 2048]
  V  = y Wv                      [2048, 256] stored as V_aug [lk, 4*(64+1)]
                                 with a ones column per head
  per head h, per lq chunk:
    S^T[lk, lq] = KT_h^T QT_h    (contraction d=64; head pairs packed at
                                  partition offsets 0/64 -> PE row tiling)
    P^T = exp(S^T)               ScalarE, PSUM -> SBUF bf16
    O^T_aug[65, lq] = sum_lk V_aug_h^T P^T   (row 64 = softmax denominators)
    O'^T = O^T * broadcast(1/sums)           DVE recip + GpSimd partition bcast
  out[lq, 1024] += O'^T_cat^T Wo  (partial; host sums the 4 head-groups)
"""

import numpy as np
import ml_dtypes

B, L, H, NH, D = 2, 2048, 1024, 16, 64
GP = 4            # head-groups (tensor-parallel factor)
CH = H // GP      # 256 local projection cols per core
HL = NH // GP     # 4 local heads
LQ = 1024         # lq chunk size
NLQ = L // LQ
NKT = L // 128    # 16 lk tiles
BF16 = ml_dtypes.bfloat16

_CACHE = {}


def _build():
    import concourse.mybir as mybir
    import concourse.tile as tile
    from concourse import bacc

    dt = mybir.dt
    f32, bf16 = dt.float32, dt.bfloat16
    Exp = mybir.ActivationFunctionType.Exp

    nc = bacc.Bacc("TRN2", target_bir_lowering=False, debug=False)
    xT = nc.declare_dram_parameter("xT", [H, L], bf16, isOutput=False)
    yT = nc.declare_dram_parameter("yT", [H, L], bf16, isOutput=False)
    wq = nc.declare_dram_parameter("wq", [H, CH], bf16, isOutput=False)
    wk = nc.declare_dram_parameter("wk", [H, CH], bf16, isOutput=False)
    wv = nc.declare_dram_parameter("wv", [H, CH], bf16, isOutput=False)
    wo = nc.declare_dram_parameter("wo", [CH, H], bf16, isOutput=False)
    out = nc.declare_dram_parameter("out", [L, H], f32, isOutput=True)

    with tile.TileContext(nc) as tc:
        with (
            tc.tile_pool(name="w", bufs=1) as wpool,
            tc.tile_pool(name="acts", bufs=1) as apool,
            tc.tile_pool(name="psA", bufs=2, space="PSUM") as psA,
            tc.tile_pool(name="psO", bufs=2, space="PSUM") as psO,
            tc.tile_pool(name="pt", bufs=18) as ptpool,
            tc.tile_pool(name="oT", bufs=2) as otpool,
            tc.tile_pool(name="sm", bufs=2) as smpool,
            tc.tile_pool(name="osb", bufs=4) as opool,
        ):
            # prefetch the exp activation table while input DMAs run
            dummy = smpool.tile([1, 8], f32, tag="dummy")
            nc.vector.memset(dummy, 0.0)
            nc.scalar.activation(dummy, dummy, Exp)

            # ---- input DMAs: big sprayed transfers (full queue fan-out),
            # ordered so K^T/Q^T ct0 (heads 0/1) can start earliest --------
            wk_sb = wpool.tile([128, 8, CH], bf16, tag="wk")
            nc.sync.dma_start(wk_sb, wk.rearrange("(t p) c -> p t c", p=128))
            yT_sb = apool.tile([128, 8, L], bf16, tag="yT")
            yr = yT.rearrange("(t p) l -> p t l", p=128)
            nc.sync.dma_start(yT_sb[:, :, 0:LQ], yr[:, :, 0:LQ])
            nc.sync.dma_start(yT_sb[:, :, LQ:L], yr[:, :, LQ:L])
            wq_sb = wpool.tile([128, 8, CH], bf16, tag="wq")
            nc.sync.dma_start(wq_sb, wq.rearrange("(t p) c -> p t c", p=128))
            xT_sb = apool.tile([128, 8, L], bf16, tag="xT")
            xr = xT.rearrange("(t p) l -> p t l", p=128)
            nc.sync.dma_start(xT_sb[:, :, 0:LQ], xr[:, :, 0:LQ])
            wv_sb = wpool.tile([128, 8, CH], bf16, tag="wv")
            nc.sync.dma_start(wv_sb, wv.rearrange("(t p) c -> p t c", p=128))
            nc.sync.dma_start(xT_sb[:, :, LQ:L], xr[:, :, LQ:L])
            wo_sb = wpool.tile([128, 2, H], bf16, tag="wo")
            nc.sync.dma_start(wo_sb, wo.rearrange("(t p) c -> p t c", p=128))

            qT_sb = apool.tile([128, 2, L], bf16, tag="qT")
            kT_sb = apool.tile([128, 2, L], bf16, tag="kT")
            vaug_sb = apool.tile([128, NKT, HL * 65], bf16, tag="vaug")

            def proj_group(w_sb, act_sb, dst, ct, lh, sl):
                # dst[:, ct, lh*LQ+sl*512 : +512] via one 8-matmul psum group
                ps = psA.tile([128, LQ], f32, tag="psA")
                off = lh * LQ + sl * 512
                for ht in range(8):
                    nc.tensor.matmul(
                        ps[:, 0:512],
                        lhsT=w_sb[:, ht, ct * 128:(ct + 1) * 128],
                        rhs=act_sb[:, ht, off:off + 512],
                        start=(ht == 0), stop=(ht == 7),
                    )
                nc.vector.tensor_copy(dst[:, ct, off:off + 512], ps[:, 0:512])

            def proj(w_sb, act_sb, dst, ct):
                for lh in range(L // LQ):
                    for sl in range(LQ // 512):
                        proj_group(w_sb, act_sb, dst, ct, lh, sl)

            def v_group(lkt):
                # one lk tile of V_aug[lk, 4*(64+1)] bf16 (+ones col per head)
                psv = psA.tile([128, LQ], f32, tag="psA")
                for ht in range(8):
                    nc.tensor.matmul(
                        psv[:, :CH],
                        lhsT=yT_sb[:, ht, lkt * 128:(lkt + 1) * 128],
                        rhs=wv_sb[:, ht, :],
                        start=(ht == 0), stop=(ht == 7),
                    )
                vh = vaug_sb[:, lkt, :].rearrange("p (h e) -> p h e", h=HL)
                nc.vector.tensor_copy(
                    vh[:, :, 0:64],
                    psv[:, :CH].rearrange("p (h e) -> p h e", h=HL))
                nc.vector.memset(vh[:, :, 64], 1.0)

            def s3_piece(ci, oT_sb, mt, nt):
                pso = psO.tile([128, LQ], f32, tag="psO")
                for kt in range(2):
                    nc.tensor.matmul(
                        pso[:, :512],
                        lhsT=oT_sb[:, kt, mt * 128:(mt + 1) * 128],
                        rhs=wo_sb[:, kt, nt * 512:(nt + 1) * 512],
                        start=(kt == 0), stop=(kt == 1),
                    )
                osb = opool.tile([128, 512], f32, tag="osb")
                nc.vector.tensor_copy(osb, pso[:, :512])
                nc.sync.dma_start(
                    out[ci * LQ + mt * 128:ci * LQ + (mt + 1) * 128,
                        nt * 512:(nt + 1) * 512],
                    osb)

            def s2(ci, h, oT_sb, extra=None):
                # per lk tile: S^T matmuls -> exp -> O^T accumulation, with
                # optional extra PE work interleaved to ride the exp stream
                po, ct2 = h % 2, h // 2
                psO_h = psO.tile([128, LQ], f32, tag="psO")
                for lkt in range(NKT):
                    psS = psA.tile([128, LQ], f32, tag="psA")
                    for sl in range(LQ // 512):
                        nc.tensor.matmul(
                            psS[:, sl * 512:(sl + 1) * 512],
                            lhsT=kT_sb[64 * po:64 * po + 64, ct2,
                                       lkt * 128:(lkt + 1) * 128],
                            rhs=qT_sb[64 * po:64 * po + 64, ct2,
                                      ci * LQ + sl * 512:ci * LQ + (sl + 1) * 512],
                            start=True, stop=True,
                        )
                    pt = ptpool.tile([128, LQ], bf16, tag="pt")
                    nc.scalar.activation(pt, psS, Exp)
                    if extra is not None:
                        extra(lkt)
                    for sl in range(LQ // 512):
                        nc.tensor.matmul(
                            psO_h[0:65, sl * 512:(sl + 1) * 512],
                            lhsT=vaug_sb[:, lkt, h * 65:(h + 1) * 65],
                            rhs=pt[:, sl * 512:(sl + 1) * 512],
                            start=(lkt == 0), stop=(lkt == NKT - 1),
                        )
                # copy PSUM->SBUF first (frees the psO slot in one DVE pass);
                # recip/bcast/mul then run off the PSUM critical path
                ocp = smpool.tile([64, LQ], f32, tag="ocp")
                nc.vector.tensor_copy(ocp, psO_h[0:64, :])
                sums = smpool.tile([1, LQ], f32, tag="sums")
                nc.vector.tensor_copy(sums, psO_h[64:65, :])
                # NB: reciprocal_approx_fast misbehaves when its input AP has
                # a non-zero base partition, hence the sums copy.
                recip = smpool.tile([1, LQ], f32, tag="recip")
                nc.vector.reciprocal_approx_fast(recip, sums)
                bcast = smpool.tile([64, LQ], f32, tag="bcast")
                nc.gpsimd.partition_broadcast(bcast, recip)
                nc.vector.tensor_mul(
                    oT_sb[64 * po:64 * po + 64, ct2, :], ocp, bcast)

            # ---- emission order: K/Q ct0 projections, then a continuous
            # per-lkt exp stream; V, the ct1 projections, and stage 3 ride
            # inside the stream as interleaved extra PE work ---------------
            oT = [otpool.tile([128, 2, LQ], bf16, tag="oT", name=f"oT{i}")
                  for i in range(NLQ)]
            proj(wk_sb, yT_sb, kT_sb, 0)
            proj(wq_sb, xT_sb, qT_sb, 0)

            def ct1_hook(lkt):
                # 8 projection groups (Q ct1, K ct1) over 16 lkt slots
                if lkt % 2 == 0:
                    i = lkt // 2
                    w_sb, act_sb, dst = ((wq_sb, xT_sb, qT_sb),
                                         (wk_sb, yT_sb, kT_sb))[i // 4]
                    proj_group(w_sb, act_sb, dst, 1, (i % 4) // 2, i % 2)

            def make_s3_hook(ci, oT_sb, lo, hi):
                pieces = [(mt, nt) for nt in range(2) for mt in range(LQ // 128)]
                per = (len(pieces) + (hi - lo) - 1) // (hi - lo)

                def hook(lkt):
                    if lo <= lkt < hi:
                        for j in range((lkt - lo) * per,
                                       min(((lkt - lo) + 1) * per, len(pieces))):
                            s3_piece(ci, oT_sb, *pieces[j])
                return hook

            s2(0, 0, oT[0], extra=v_group)
            s2(0, 1, oT[0], extra=ct1_hook)
            s2(0, 2, oT[0])
            s2(0, 3, oT[0])
            s2(1, 0, oT[1], extra=make_s3_hook(0, oT[0], 4, 12))
            s2(1, 1, oT[1])
            s2(1, 2, oT[1])
            s2(1, 3, oT[1])
            for mt in range(LQ // 128):
                for nt in range(2):
                    s3_piece(1, oT[1], mt, nt)
    nc.compile()
    return nc


def _get_nc():
    if "nc" not in _CACHE:
        _CACHE["nc"] = _build()
    return _CACHE["nc"]


def _in_maps(x, y, Wq, Wk, Wv, Wo):
    maps = []
    for core in range(8):
        b, g = core // GP, core % GP
        cs = slice(g * CH, (g + 1) * CH)
        maps.append({
            "xT": np.ascontiguousarray(x[b].T).astype(BF16),
            "yT": np.ascontiguousarray(y[b].T).astype(BF16),
            "wq": np.ascontiguousarray(Wq[:, cs] * np.float32(0.125)).astype(BF16),
            "wk": np.ascontiguousarray(Wk[:, cs]).astype(BF16),
            "wv": np.ascontiguousarray(Wv[:, cs]).astype(BF16),
            "wo": np.ascontiguousarray(Wo[cs, :]).astype(BF16),
        })
    return maps


def _install_ntff_hook():
    """Provide the antenv.axon_hooks shim missing from this container so
    run_bass_kernel_spmd(trace=True) can drive NTFF profiling via ctypes."""
    import sys
    import types
    try:
        from antenv.axon_hooks import get_axon_ntff_profile_hook  # noqa: F401
        return
    except ImportError:
        pass
    from trn_agent_boot.trn_boot import _ntff_profile_via_ctypes
    hook = _ntff_profile_via_ctypes("/opt/axon/libaxon_pjrt.so")
    mod = types.ModuleType("antenv.axon_hooks")
    mod.get_axon_ntff_profile_hook = lambda: hook
    mod.set_axon_ntff_profile_hook = lambda h: None
    sys.modules["antenv.axon_hooks"] = mod


def _run(inputs, trace=False):
    from concourse import bass_utils

    if trace:
        _install_ntff_hook()

    x, y, bias = inputs["x"], inputs["y"], inputs["bias"]
    if np.count_nonzero(np.asarray(bias)):
        raise NotImplementedError("nonzero attention bias not supported")
    nc = _get_nc()
    maps = _in_maps(np.asarray(x, np.float32), np.asarray(y, np.float32),
                    np.asarray(inputs["Wq"], np.float32),
                    np.asarray(inputs["Wk"], np.float32),
                    np.asarray(inputs["Wv"], np.float32),
                    np.asarray(inputs["Wo"], np.float32))
    res = bass_utils.run_bass_kernel_spmd(
        nc, maps, list(range(8)), trace=trace)
    out = np.zeros((B, L, H), np.float32)
    for core in range(8):
        out[core // GP] += res.results[core]["out"]
    return out, res


def kernel(**inputs):
    out, _ = _run(inputs, trace=False)
    return out


# revision 20
# speedup vs baseline: 1.1534x; 1.0089x over previous
"""Multi-head attention (B=2, L=2048, H=1024, NH=16) on 8 TRN2 NeuronCores.

Sharding: data-parallel over batch (2) x tensor-parallel over heads (4 groups
of 4 heads).  core = b*4 + g handles batch b, heads [4g, 4g+4).  Wq/Wk/Wv are
split column-wise, Wo row-wise; each core produces a partial [L, H] output
that the host sums per batch (the row-parallel all-reduce done host-side).

Device math (per core), all matmuls bf16 inputs / fp32 PSUM accumulation:
  QT = (Wq*0.125)^T x^T          [256, 2048]  (softmax scale folded into Wq)
  KT = Wk^T y^T                  [256, 2048]
  V  = y Wv                      [2048, 256] stored as V_aug [lk, 4*(64+1)]
                                 with a ones column per head
  per head h, per lq chunk:
    S^T[lk, lq] = KT_h^T QT_h    (contraction d=64; head pairs packed at
                                  partition offsets 0/64 -> PE row tiling)
    P^T = exp(S^T)               ScalarE, PSUM -> SBUF bf16
    O^T_aug[65, lq] = sum_lk V_aug_h^T P^T   (row 64 = softmax denominators)
    O'^T = O^T * broadcast(1/sums)           DVE recip + GpSimd partition bcast
  out[lq, 1024] += O'^T_cat^T Wo  (partial; host sums the 4 head-groups)
"""

import numpy as np
import ml_dtypes

B, L, H, NH, D = 2, 2048, 1024, 16, 64
GP = 4            # head-groups (tensor-parallel factor)
CH = H // GP      # 256 local projection cols per core
HL = NH // GP     # 4 local heads
LQ = 1024         # lq chunk size
NLQ = L // LQ
NKT = L // 128    # 16 lk tiles
BF16 = ml_dtypes.bfloat16

_CACHE = {}


def _build():
    import concourse.mybir as mybir
    import concourse.tile as tile
    from concourse import bacc

    dt = mybir.dt
    f32, bf16 = dt.float32, dt.bfloat16
    Exp = mybir.ActivationFunctionType.Exp

    nc = bacc.Bacc("TRN2", target_bir_lowering=False, debug=False)
    xT = nc.declare_dram_parameter("xT", [H, L], bf16, isOutput=False)
    yT = nc.declare_dram_parameter("yT", [H, L], bf16, isOutput=False)
    wq = nc.declare_dram_parameter("wq", [H, CH], bf16, isOutput=False)
    wk = nc.declare_dram_parameter("wk", [H, CH], bf16, isOutput=False)
    wv = nc.declare_dram_parameter("wv", [H, CH], bf16, isOutput=False)
    wo = nc.declare_dram_parameter("wo", [CH, H], bf16, isOutput=False)
    out = nc.declare_dram_parameter("out", [L, H], f32, isOutput=True)

    with tile.TileContext(nc) as tc:
        with (
            tc.tile_pool(name="w", bufs=1) as wpool,
            tc.tile_pool(name="acts", bufs=1) as apool,
            tc.tile_pool(name="psA", bufs=2, space="PSUM") as psA,
            tc.tile_pool(name="psO", bufs=2, space="PSUM") as psO,
            tc.tile_pool(name="pt", bufs=6) as ptpool,
            tc.tile_pool(name="oT", bufs=2) as otpool,
            tc.tile_pool(name="sm", bufs=2) as smpool,
            tc.tile_pool(name="osb", bufs=4) as opool,
        ):
            # prefetch the exp activation table while input DMAs run
            dummy = smpool.tile([1, 8], f32, tag="dummy")
            nc.vector.memset(dummy, 0.0)
            nc.scalar.activation(dummy, dummy, Exp)

            # ---- input DMAs: weights first (small), then activations in
            # 512-column chunks consumed by projection groups as they land -
            wk_sb = wpool.tile([128, 8, CH], bf16, tag="wk")
            nc.sync.dma_start(wk_sb, wk.rearrange("(t p) c -> p t c", p=128))
            wq_sb = wpool.tile([128, 8, CH], bf16, tag="wq")
            nc.sync.dma_start(wq_sb, wq.rearrange("(t p) c -> p t c", p=128))
            wv_sb = wpool.tile([128, 8, CH], bf16, tag="wv")
            nc.sync.dma_start(wv_sb, wv.rearrange("(t p) c -> p t c", p=128))
            yT_sb = apool.tile([128, 8, L], bf16, tag="yT")
            yr = yT.rearrange("(t p) l -> p t l", p=128)
            for c in range(4):
                nc.sync.dma_start(yT_sb[:, :, c * 512:(c + 1) * 512],
                                  yr[:, :, c * 512:(c + 1) * 512])
            xT_sb = apool.tile([128, 8, L], bf16, tag="xT")
            xr = xT.rearrange("(t p) l -> p t l", p=128)
            for c in range(4):
                nc.sync.dma_start(xT_sb[:, :, c * 512:(c + 1) * 512],
                                  xr[:, :, c * 512:(c + 1) * 512])
            wo_sb = wpool.tile([128, 2, H], bf16, tag="wo")
            nc.sync.dma_start(wo_sb, wo.rearrange("(t p) c -> p t c", p=128))

            qT_sb = apool.tile([128, 2, L], bf16, tag="qT")
            kT_sb = apool.tile([128, 2, L], bf16, tag="kT")
            vaug_sb = apool.tile([128, NKT, HL * 65], bf16, tag="vaug")

            def proj_group(w_sb, act_sb, dst, ct, lh, sl):
                # dst[:, ct, lh*LQ+sl*512 : +512] via one 8-matmul psum group
                ps = psA.tile([128, LQ], f32, tag="psA")
                off = lh * LQ + sl * 512
                for ht in range(8):
                    nc.tensor.matmul(
                        ps[:, 0:512],
                        lhsT=w_sb[:, ht, ct * 128:(ct + 1) * 128],
                        rhs=act_sb[:, ht, off:off + 512],
                        start=(ht == 0), stop=(ht == 7),
                    )
                nc.vector.tensor_copy(dst[:, ct, off:off + 512], ps[:, 0:512])

            def v_group(lkt):
                # one lk tile of V_aug[lk, 4*(64+1)] bf16 (+ones col per head)
                psv = psA.tile([128, LQ], f32, tag="psA")
                for ht in range(8):
                    nc.tensor.matmul(
                        psv[:, :CH],
                        lhsT=yT_sb[:, ht, lkt * 128:(lkt + 1) * 128],
                        rhs=wv_sb[:, ht, :],
                        start=(ht == 0), stop=(ht == 7),
                    )
                vh = vaug_sb[:, lkt, :].rearrange("p (h e) -> p h e", h=HL)
                nc.vector.tensor_copy(
                    vh[:, :, 0:64],
                    psv[:, :CH].rearrange("p (h e) -> p h e", h=HL))
                nc.vector.memset(vh[:, :, 64], 1.0)

            def s3_piece(ci, oT_sb, mt, nt):
                pso = psO.tile([128, LQ], f32, tag="psO")
                for kt in range(2):
                    nc.tensor.matmul(
                        pso[:, :512],
                        lhsT=oT_sb[:, kt, mt * 128:(mt + 1) * 128],
                        rhs=wo_sb[:, kt, nt * 512:(nt + 1) * 512],
                        start=(kt == 0), stop=(kt == 1),
                    )
                osb = opool.tile([128, 512], f32, tag="osb")
                nc.vector.tensor_copy(osb, pso[:, :512])
                nc.sync.dma_start(
                    out[ci * LQ + mt * 128:ci * LQ + (mt + 1) * 128,
                        nt * 512:(nt + 1) * 512],
                    osb)

            def s2(ci, h, oT_sb, extra=None):
                # per lk tile: S^T matmuls -> exp -> O^T accumulation, with
                # optional extra PE work interleaved to ride the exp stream
                po, ct2 = h % 2, h // 2
                psO_h = psO.tile([128, LQ], f32, tag="psO")
                for lkt in range(NKT):
                    psS = psA.tile([128, LQ], f32, tag="psA")
                    for sl in range(LQ // 512):
                        nc.tensor.matmul(
                            psS[:, sl * 512:(sl + 1) * 512],
                            lhsT=kT_sb[64 * po:64 * po + 64, ct2,
                                       lkt * 128:(lkt + 1) * 128],
                            rhs=qT_sb[64 * po:64 * po + 64, ct2,
                                      ci * LQ + sl * 512:ci * LQ + (sl + 1) * 512],
                            start=True, stop=True,
                        )
                    pt = ptpool.tile([128, LQ], bf16, tag="pt")
                    nc.scalar.activation(pt, psS, Exp)
                    if extra is not None:
                        extra(lkt)
                    for sl in range(LQ // 512):
                        nc.tensor.matmul(
                            psO_h[0:65, sl * 512:(sl + 1) * 512],
                            lhsT=vaug_sb[:, lkt, h * 65:(h + 1) * 65],
                            rhs=pt[:, sl * 512:(sl + 1) * 512],
                            start=(lkt == 0), stop=(lkt == NKT - 1),
                        )
                # copy PSUM->SBUF first (frees the psO slot in one DVE pass);
                # recip/bcast/mul then run off the PSUM critical path
                ocp = smpool.tile([64, LQ], f32, tag="ocp")
                nc.vector.tensor_copy(ocp, psO_h[0:64, :])
                sums = smpool.tile([1, LQ], f32, tag="sums")
                nc.vector.tensor_copy(sums, psO_h[64:65, :])
                # NB: reciprocal_approx_fast misbehaves when its input AP has
                # a non-zero base partition, hence the sums copy.
                recip = smpool.tile([1, LQ], f32, tag="recip")
                nc.vector.reciprocal_approx_fast(recip, sums)
                bcast = smpool.tile([64, LQ], f32, tag="bcast")
                nc.gpsimd.partition_broadcast(bcast, recip)
                nc.vector.tensor_mul(
                    oT_sb[64 * po:64 * po + 64, ct2, :], ocp, bcast)

            # ---- emission order: pack the DMA-bound startup window with
            # K^T ct0 / Q^T ct0(lh0) projections plus half of V, then run a
            # continuous per-lkt exp stream; remaining projections and
            # stage 3 ride inside the stream as interleaved extra work -----
            oT = [otpool.tile([128, 2, LQ], bf16, tag="oT", name=f"oT{i}")
                  for i in range(NLQ)]
            # startup: interleaved with yT/xT chunk arrival
            proj_group(wk_sb, yT_sb, kT_sb, 0, 0, 0)
            for j in range(4):
                v_group(j)
            proj_group(wk_sb, yT_sb, kT_sb, 0, 0, 1)
            for j in range(4, 8):
                v_group(j)
            proj_group(wk_sb, yT_sb, kT_sb, 0, 1, 0)
            proj_group(wk_sb, yT_sb, kT_sb, 0, 1, 1)
            proj_group(wq_sb, xT_sb, qT_sb, 0, 0, 0)
            proj_group(wq_sb, xT_sb, qT_sb, 0, 0, 1)

            def make_hook(jobs, slots):
                # spread thunks over the given lkt slots (one per slot)
                sched = dict(zip(slots, jobs))

                def hook(lkt):
                    if lkt in sched:
                        sched[lkt]()
                return hook

            vj = [(lambda j=j: v_group(j)) for j in range(8, NKT)]
            pj = lambda w, a, d, ct, lh, sl: (  # noqa: E731
                lambda: proj_group(w, a, d, ct, lh, sl))
            h0_hook = make_hook(vj, [0, 2, 4, 6, 8, 10, 12, 14])
            h1_hook = make_hook(
                [pj(wk_sb, yT_sb, kT_sb, 1, 0, 0),
                 pj(wk_sb, yT_sb, kT_sb, 1, 0, 1),
                 pj(wk_sb, yT_sb, kT_sb, 1, 1, 0),
                 pj(wk_sb, yT_sb, kT_sb, 1, 1, 1),
                 pj(wq_sb, xT_sb, qT_sb, 1, 0, 0),
                 pj(wq_sb, xT_sb, qT_sb, 1, 0, 1)],
                [0, 3, 6, 9, 12, 14])
            h2_hook = make_hook(
                [pj(wq_sb, xT_sb, qT_sb, 0, 1, 0),
                 pj(wq_sb, xT_sb, qT_sb, 0, 1, 1),
                 pj(wq_sb, xT_sb, qT_sb, 1, 1, 0),
                 pj(wq_sb, xT_sb, qT_sb, 1, 1, 1)],
                [0, 4, 8, 12])
            s3_jobs = [(lambda mt=mt, nt=nt: s3_piece(0, oT[0], mt, nt))
                       for nt in range(2) for mt in range(LQ // 128)]

            def make_s3_hook(lo, hi):
                # two pieces per lkt over [lo, hi)
                it = iter(s3_jobs)

                def hook(lkt):
                    if lo <= lkt < hi:
                        for _ in range(2):
                            j = next(it, None)
                            if j is not None:
                                j()
                return hook

            s2(0, 0, oT[0], extra=h0_hook)
            s2(0, 1, oT[0], extra=h1_hook)
            s2(0, 2, oT[0], extra=h2_hook)
            s2(0, 3, oT[0])
            s2(1, 0, oT[1], extra=make_s3_hook(4, 12))
            s2(1, 1, oT[1])
            s2(1, 2, oT[1])
            s2(1, 3, oT[1])
            for mt in range(LQ // 128):
                for nt in range(2):
                    s3_piece(1, oT[1], mt, nt)
    nc.compile()
    return nc


def _get_nc():
    if "nc" not in _CACHE:
        _CACHE["nc"] = _build()
    return _CACHE["nc"]


def _in_maps(x, y, Wq, Wk, Wv, Wo):
    maps = []
    for core in range(8):
        b, g = core // GP, core % GP
        cs = slice(g * CH, (g + 1) * CH)
        maps.append({
            "xT": np.ascontiguousarray(x[b].T).astype(BF16),
            "yT": np.ascontiguousarray(y[b].T).astype(BF16),
            "wq": np.ascontiguousarray(Wq[:, cs] * np.float32(0.125)).astype(BF16),
            "wk": np.ascontiguousarray(Wk[:, cs]).astype(BF16),
            "wv": np.ascontiguousarray(Wv[:, cs]).astype(BF16),
            "wo": np.ascontiguousarray(Wo[cs, :]).astype(BF16),
        })
    return maps


def _install_ntff_hook():
    """Provide the antenv.axon_hooks shim missing from this container so
    run_bass_kernel_spmd(trace=True) can drive NTFF profiling via ctypes."""
    import sys
    import types
    try:
        from antenv.axon_hooks import get_axon_ntff_profile_hook  # noqa: F401
        return
    except ImportError:
        pass
    from trn_agent_boot.trn_boot import _ntff_profile_via_ctypes
    hook = _ntff_profile_via_ctypes("/opt/axon/libaxon_pjrt.so")
    mod = types.ModuleType("antenv.axon_hooks")
    mod.get_axon_ntff_profile_hook = lambda: hook
    mod.set_axon_ntff_profile_hook = lambda h: None
    sys.modules["antenv.axon_hooks"] = mod


def _run(inputs, trace=False):
    from concourse import bass_utils

    if trace:
        _install_ntff_hook()

    x, y, bias = inputs["x"], inputs["y"], inputs["bias"]
    if np.count_nonzero(np.asarray(bias)):
        raise NotImplementedError("nonzero attention bias not supported")
    nc = _get_nc()
    maps = _in_maps(np.asarray(x, np.float32), np.asarray(y, np.float32),
                    np.asarray(inputs["Wq"], np.float32),
                    np.asarray(inputs["Wk"], np.float32),
                    np.asarray(inputs["Wv"], np.float32),
                    np.asarray(inputs["Wo"], np.float32))
    res = bass_utils.run_bass_kernel_spmd(
        nc, maps, list(range(8)), trace=trace)
    out = np.zeros((B, L, H), np.float32)
    for core in range(8):
        out[core // GP] += res.results[core]["out"]
    return out, res


def kernel(**inputs):
    out, _ = _run(inputs, trace=False)
    return out
